# revision 25
# baseline (speedup 1.0000x reference)
"""MoE router kernel for Trainium2 (8 NeuronCores, SPMD data-parallel).

Computes, for x [B,S,H] and gate_w [E,H]:
    logits = x @ gate_w.T           # [B,S,E]
    p = softmax(logits, -1)
    w, i = top_k(p, 2); w = w / w.sum(-1, keepdims=True)
with w1 = sigmoid(l1 - l2), w2 = sigmoid(l2 - l1) (renormalized top-2
softmax collapses to a sigmoid of the top-2 logit gap).

v9 layout: x is transposed to [h, tok], cast to fp16 on the host
(halves HBM traffic; ~11 index flips, rel 1.3e-2 < 2e-2 gate), and
sharded TOKEN-MAJOR: each 1 MiB DMA carries 8 h-chunks x 512 tokens of
ONE token block, so block s finishes its h-accumulation at (s+1)/4 of
the stream and its merge/top-k overlaps the next block's DMAs. Only
block 3's last group + backend remain after the last input byte.

The gate weight keeps ~fp32 precision via a hi+lo fp16 split packed
into one 128-wide stationary [w_hi | w_lo] (E=64 fills only half the
PE array, so the correction columns ride along for free); it is folded
into block-0's DMAs (+32KB per chunk-group) and later blocks read it
from SBUF. The merge matmul lg[tok,e] = ltT_blk.T @ [I64; I64] folds
the hi+lo sum AND the back-transpose into one PE op per 128 tokens.
Merges are deferred one block so the PSUM->SBUF copy hides behind the
next block's first GEMM group.
"""

import sys

sys.path.insert(0, "/opt/trn_rl_repo")

import numpy as np

import concourse.bass as bass
import concourse.mybir as mybir
import concourse.tile as tile
from concourse.bass_utils import run_bass_kernel_spmd
import orjson
import concourse.bass_utils as _bu
import concourse.bass2jax as _b2j

_orig_compile_bir = _bu.compile_bir_kernel


def _legalize_waits(bir_json: bytes) -> bytes:
    """This walrus build allows only ONE sync-wait per compute
    instruction; move excess waits onto a Drain inserted just before
    (Drain accepts many waits)."""
    m = orjson.loads(bir_json)
    changed = False
    for fn in m["functions"]:
        for blk in fn["blocks"]:
            out = []
            for inst in blk["instructions"]:
                si = inst.get("sync_info")
                w = (si or {}).get("on_wait") or []
                if len(w) > 1:
                    for k, wk in enumerate(w[:-1]):
                        out.append({
                            "debug": inst.get("debug", 0),
                            "engine": inst["engine"],
                            "ins": [], "outs": [],
                            "name": inst["name"] + f"-lw{k}",
                            "opcode": "Drain",
                            "sync_info": {"on_update": [], "on_wait": [wk]},
                        })
                    si["on_wait"] = w[-1:]
                    changed = True
                out.append(inst)
            blk["instructions"] = out
    return orjson.dumps(m) if changed else bir_json


def _compile_bir_legalized(bir_json, tmpdir, neff_name="file.neff"):
    return _orig_compile_bir(_legalize_waits(bir_json), tmpdir, neff_name)


_bu.compile_bir_kernel = _compile_bir_legalized
_b2j.compile_bir_kernel = _compile_bir_legalized

F32 = mybir.dt.float32
F16 = mybir.dt.float16
U32 = mybir.dt.uint32

B, S, H, E = 4, 4096, 4096, 64
N_CORES = 8
P = 128                      # partitions / tile height
TOK_TOTAL = B * S            # 16384
TOK = TOK_TOTAL // N_CORES   # 2048 tokens per core
NCH = H // P                 # 32 contraction chunks of 128
NT = TOK // P                # 16 token tiles per core
NB = 4                       # token blocks per core
BW = TOK // NB               # 512 tokens per block
NG = 4                       # chunk groups per block
GC = NCH // NG               # 8 chunks per group
TPB = BW // P                # merge matmuls (128-token tiles) per block


def build_nc():
    """Build the per-core Bass program (SPMD: same program, 8 cores)."""
    nc = bass.Bass()

    # block 0's DMAs carry x + the folded (hi|lo) gate weight
    x0_ext = nc.declare_dram_parameter("x0", [NG, P, GC, BW + P], F16,
                                       isOutput=False)
    x_ext = nc.declare_dram_parameter("x", [(NB - 1) * NG, P, GC, BW],
                                      F16, isOutput=False)
    m_ext = nc.declare_dram_parameter("mrg", [P, E], F32, isOutput=False)
    ow_ext = nc.declare_dram_parameter("out_w", [P, NT, 2], F32,
                                       isOutput=True)
    oi_ext = nc.declare_dram_parameter("out_i", [P, NT, 2], U32,
                                       isOutput=True)

    with tile.TileContext(nc) as tc:
        with (
            tc.tile_pool(name="consts", bufs=1) as consts,
            tc.tile_pool(name="xin0", bufs=NG) as xpool0,
            tc.tile_pool(name="xin", bufs=(NB - 1) * NG) as xpool,
            tc.tile_pool(name="ps_lt", bufs=2, space="PSUM") as ps_lt,
            tc.tile_pool(name="ps_lg", bufs=2, space="PSUM") as ps_lg,
            tc.tile_pool(name="ps_misc", bufs=2, space="PSUM") as ps_misc,
            tc.tile_pool(name="work", bufs=2) as work,
            tc.tile_pool(name="outp", bufs=1) as outp,
        ):
            # mrg tile; DMA'd mid-ring below. NOT via gpsimd SWDGE -
            # any SWDGE DMA degrades SDMA engine 15 for the whole run
            # (descriptor-ring AXI contention).
            m_sb = consts.tile([P, E], F32)

            # Primers: walrus allows only ONE sync-wait per compute
            # instruction. Give every engine a first op with no other
            # dependency, and absorb the mrg-DMA sem into a throwaway
            # PE op.
            prim = consts.tile([P, 4], F32)
            nc.scalar.copy(prim[:, 1:2], nc.const_aps.tensor(1.0, (P, 1)))
            nc.gpsimd.memset(prim[:, 2:3], 0.0)
            # PE warm-up burst on a memset scratch so the HAM clock-gate
            # reaches K=8/8 before the real GEMM starts; alternate two
            # PSUM banks (same-bank WAW serializes the PE).
            wsc = consts.tile([P, 4 * P], F16)
            nc.vector.memset(wsc[:], 0.0)
            scr = ps_misc.tile([P, BW], F32)
            scr2 = ps_misc.tile([P, BW], F32)
            for k in range(8):
                nc.tensor.matmul([scr, scr2][k % 2][:], wsc[:, 0:P],
                                 wsc[:], start=True, stop=True)

            # all 16 input DMAs up front, alternating the two HWDGE
            # rings; both rings open with x (a const DMA ahead of the
            # first x delays it ~10us: data + ring completion gap).
            # mrg (32KB) slots in after sync's first x: lands ~14us,
            # needed at ~21us, and its ring gap hides behind the other
            # ring's data mid-stream.
            xts = {}
            for s in range(NB):
                for g in range(NG):
                    j = NG * s + g
                    if s == 0:
                        xt = xpool0.tile([P, GC, BW + P], F16, name="x0t",
                                         tag="x0t")
                        src = x0_ext[g]
                    else:
                        xt = xpool.tile([P, GC, BW], F16, name="xt",
                                        tag="xt")
                        src = x_ext[j - NG]
                    dma = nc.scalar.dma_start if j % 2 == 0 \
                        else nc.sync.dma_start
                    dma(xt[:], src)
                    xts[(s, g)] = xt
                    if j == 1:
                        nc.sync.dma_start(m_sb[:], m_ext[:])

            def wsl(c):
                # (hi|lo) stationary for chunk c, folded in block-0 tiles
                return xts[(0, c // GC)][:, c % GC, BW:BW + P]

            mx_all = outp.tile([P, NT, 8], F32)
            ix_all = outp.tile([P, NT, 8], U32)
            gap = outp.tile([P, NT, 1], F32)
            ow_all = outp.tile([P, NT, 2], F32)
            oi_all = outp.tile([P, NT, 2], U32)

            lt_sbs = {}

            def backend_copy(s, lt_ps):
                # two half copies: the first merge pair only needs the
                # first 256 columns
                lt_sb = work.tile([P, BW], F32, name="lt_sb", tag="lt_sb")
                hw = BW // 2
                nc.scalar.copy(lt_sb[:, 0:hw], lt_ps[:, 0:hw])
                nc.scalar.copy(lt_sb[:, hw:BW], lt_ps[:, hw:BW])
                lt_sbs[s] = lt_sb

            def backend_rest(s):
                # merge+transpose: lg[tok, e] = ltT_blk.T @ [I64; I64]
                lt_sb = lt_sbs.pop(s)
                # alternate merge target banks: independent groups in
                # the SAME bank serialize (WAW on the bank clear); two
                # banks pipeline and each max8 starts after its own pair
                lgs = [ps_lg.tile([P, TPB // 2, E], F32, name="lg",
                                  tag="lg") for _ in range(2)]
                for b in range(TPB):
                    nc.tensor.matmul(
                        lgs[b % 2][:, b // 2, :],
                        lt_sb[:, b * P:(b + 1) * P],
                        m_sb[:],
                        start=(b < 2), stop=(b >= TPB - 2),
                    )
                for b in range(TPB):
                    t = s * TPB + b
                    nc.vector.max(mx_all[:, t, :], lgs[b % 2][:, b // 2, :])
                sl = slice(s * TPB, (s + 1) * TPB)
                # weights path (ACT) only needs the max VALUES; emit it
                # before the index chain so the two run in parallel
                nc.vector.scalar_tensor_tensor(
                    gap[:, sl, :], mx_all[:, sl, 0:1], 1.0,
                    mx_all[:, sl, 1:2],
                    op0=mybir.AluOpType.mult, op1=mybir.AluOpType.subtract,
                )
                nc.scalar.activation(
                    ow_all[:, sl, 0:1], gap[:, sl, :],
                    mybir.ActivationFunctionType.Sigmoid,
                )
                nc.scalar.activation(
                    ow_all[:, sl, 1:2], gap[:, sl, :],
                    mybir.ActivationFunctionType.Sigmoid, scale=-1.0,
                )
                for b in range(TPB):
                    t = s * TPB + b
                    nc.vector.max_index(ix_all[:, t, :], mx_all[:, t, :],
                                        lgs[b % 2][:, b // 2, :])
                nc.gpsimd.tensor_copy(oi_all[:, sl, :], ix_all[:, sl, 0:2])

            # per-block GEMM chase: block s's h-accumulation completes at
            # (s+1)/4 of the stream; its backend overlaps block s+1's
            # DMAs/GEMMs. Merges deferred one block so the PSUM->SBUF
            # copy hides behind the next block's first GEMM group.
            for s in range(NB):
                lt_ps = ps_lt.tile([P, BW], F32, name="lt", tag="lt")
                for g in range(NG):
                    for k in range(GC):
                        nc.tensor.matmul(
                            lt_ps[:],
                            wsl(GC * g + k),
                            xts[(s, g)][:, k, 0:BW],
                            start=(g == 0 and k == 0),
                            stop=(g == NG - 1 and k == GC - 1),
                        )
                    if g == 0 and s > 0:
                        backend_rest(s - 1)
                backend_copy(s, lt_ps)
            backend_rest(NB - 1)

            nc.sync.dma_start(ow_ext[:], ow_all[:])
            nc.scalar.dma_start(oi_ext[:], oi_all[:])

    return nc


_NC_CACHE = {}


def _get_nc():
    if "nc" not in _NC_CACHE:
        _NC_CACHE["nc"] = build_nc()
    return _NC_CACHE["nc"]


def make_in_maps(x: np.ndarray, gate_w: np.ndarray):
    """Shard full inputs into per-core input maps (host-side layout +
    fp16 cast; not on the device critical path)."""
    xf = x.reshape(TOK_TOTAL, H)
    # [core, blk, tok, grp, chk, p] -> [core, blk, grp, p, chk, tok]
    # h = g*(GC*P) + k*P + p
    xq = xf.reshape(N_CORES, NB, BW, NG, GC, P).astype(np.float16)
    xq = xq.transpose(0, 1, 3, 5, 4, 2)
    # gate weight hi/lo fp16 split: whl[p, c, 0:64]=hi, [p, c, 64:128]=lo
    w_hi = gate_w.astype(np.float16)
    w_lo = (gate_w - w_hi.astype(np.float32)).astype(np.float16)
    wh = w_hi.T.reshape(NCH, P, E).transpose(1, 0, 2)
    wl = w_lo.T.reshape(NCH, P, E).transpose(1, 0, 2)
    whl = np.concatenate([wh, wl], axis=2)          # [P, NCH, P]
    wg = whl.reshape(P, NG, GC, P).transpose(1, 0, 2, 3)  # [NG, P, GC, P]
    mrg = np.ascontiguousarray(
        np.vstack([np.eye(E), np.eye(E)]).astype(np.float32))
    return [
        {"x0": np.ascontiguousarray(
            np.concatenate([xq[i, 0], wg], axis=3)),
         "x": np.ascontiguousarray(
             xq[i, 1:].reshape((NB - 1) * NG, P, GC, BW)),
         "mrg": mrg}
        for i in range(N_CORES)
    ]


def kernel(x, gate_w, _trace: bool = False):
    x = np.asarray(x, dtype=np.float32)
    gate_w = np.asarray(gate_w, dtype=np.float32)
    nc = _get_nc()
    in_maps = make_in_maps(x, gate_w)
    res = run_bass_kernel_spmd(
        nc, in_maps, core_ids=list(range(N_CORES)), trace=_trace
    )
    out_w = np.concatenate(
        [res.results[i]["out_w"].transpose(1, 0, 2).reshape(TOK, 2)
         for i in range(N_CORES)])
    out_i = np.concatenate(
        [res.results[i]["out_i"].transpose(1, 0, 2).reshape(TOK, 2)
         for i in range(N_CORES)])
    topk_weights = out_w.reshape(B, S, 2)
    topk_indices = out_i.astype(np.int32).reshape(B, S, 2)
    if _trace:
        kernel._last_result = res
    return topk_weights, topk_indices


# revision 28
# speedup vs baseline: 1.0970x; 1.0970x over previous
"""MoE router kernel for Trainium2 (8 NeuronCores, SPMD data-parallel).

Computes, for x [B,S,H] and gate_w [E,H]:
    logits = x @ gate_w.T           # [B,S,E]
    p = softmax(logits, -1)
    w, i = top_k(p, 2); w = w / w.sum(-1, keepdims=True)
with w1 = sigmoid(l1 - l2), w2 = sigmoid(l2 - l1) (renormalized top-2
softmax collapses to a sigmoid of the top-2 logit gap).

v9 layout: x is transposed to [h, tok], cast to fp16 on the host
(halves HBM traffic; ~11 index flips, rel 1.3e-2 < 2e-2 gate), and
sharded TOKEN-MAJOR: each 1 MiB DMA carries 8 h-chunks x 512 tokens of
ONE token block, so block s finishes its h-accumulation at (s+1)/4 of
the stream and its merge/top-k overlaps the next block's DMAs. Only
block 3's last group + backend remain after the last input byte.

The gate weight keeps ~fp32 precision via a hi+lo fp16 split packed
into one 128-wide stationary [w_hi | w_lo] (E=64 fills only half the
PE array, so the correction columns ride along for free); it is folded
into block-0's DMAs (+32KB per chunk-group) and later blocks read it
from SBUF. The merge matmul lg[tok,e] = ltT_blk.T @ [I64; I64] folds
the hi+lo sum AND the back-transpose into one PE op per 128 tokens.
Merges are deferred one block so the PSUM->SBUF copy hides behind the
next block's first GEMM group.
"""

import sys

sys.path.insert(0, "/opt/trn_rl_repo")

import numpy as np

import concourse.bass as bass
import concourse.mybir as mybir
import concourse.tile as tile
from concourse.bass_utils import run_bass_kernel_spmd
import orjson
import concourse.bass_utils as _bu
import concourse.bass2jax as _b2j

_orig_compile_bir = _bu.compile_bir_kernel


def _legalize_waits(bir_json: bytes) -> bytes:
    """This walrus build allows only ONE sync-wait per compute
    instruction; move excess waits onto a Drain inserted just before
    (Drain accepts many waits)."""
    m = orjson.loads(bir_json)
    changed = False
    for fn in m["functions"]:
        for blk in fn["blocks"]:
            out = []
            for inst in blk["instructions"]:
                si = inst.get("sync_info")
                w = (si or {}).get("on_wait") or []
                if len(w) > 1:
                    for k, wk in enumerate(w[:-1]):
                        out.append({
                            "debug": inst.get("debug", 0),
                            "engine": inst["engine"],
                            "ins": [], "outs": [],
                            "name": inst["name"] + f"-lw{k}",
                            "opcode": "Drain",
                            "sync_info": {"on_update": [], "on_wait": [wk]},
                        })
                    si["on_wait"] = w[-1:]
                    changed = True
                out.append(inst)
            blk["instructions"] = out
    return orjson.dumps(m) if changed else bir_json


def _compile_bir_legalized(bir_json, tmpdir, neff_name="file.neff"):
    return _orig_compile_bir(_legalize_waits(bir_json), tmpdir, neff_name)


_bu.compile_bir_kernel = _compile_bir_legalized
_b2j.compile_bir_kernel = _compile_bir_legalized

F32 = mybir.dt.float32
F16 = mybir.dt.float16
U32 = mybir.dt.uint32

B, S, H, E = 4, 4096, 4096, 64
N_CORES = 8
P = 128                      # partitions / tile height
TOK_TOTAL = B * S            # 16384
TOK = TOK_TOTAL // N_CORES   # 2048 tokens per core
NCH = H // P                 # 32 contraction chunks of 128
NT = TOK // P                # 16 token tiles per core
NB = 4                       # token blocks per core
BW = TOK // NB               # 512 tokens per block
NG = 4                       # chunk groups per block
GC = NCH // NG               # 8 chunks per group
TPB = BW // P                # merge matmuls (128-token tiles) per block


def build_nc():
    """Build the per-core Bass program (SPMD: same program, 8 cores)."""
    nc = bass.Bass()

    # block 0's DMAs carry x + the folded (hi|lo) gate weight
    x0_ext = nc.declare_dram_parameter("x0", [NG, P, GC, BW + P], F16,
                                       isOutput=False)
    x_ext = nc.declare_dram_parameter("x", [(NB - 1) * NG, P, GC, BW],
                                      F16, isOutput=False)
    m_ext = nc.declare_dram_parameter("mrg", [P, E], F32, isOutput=False)
    ow_ext = nc.declare_dram_parameter("out_w", [P, NT, 2], F32,
                                       isOutput=True)
    oi_ext = nc.declare_dram_parameter("out_i", [P, NT, 2], U32,
                                       isOutput=True)

    with tile.TileContext(nc) as tc:
        with (
            tc.tile_pool(name="consts", bufs=1) as consts,
            tc.tile_pool(name="xin0", bufs=NG) as xpool0,
            tc.tile_pool(name="xin", bufs=(NB - 1) * NG) as xpool,
            tc.tile_pool(name="ps_lt", bufs=2, space="PSUM") as ps_lt,
            tc.tile_pool(name="ps_lg", bufs=2, space="PSUM") as ps_lg,
            tc.tile_pool(name="ps_misc", bufs=2, space="PSUM") as ps_misc,
            tc.tile_pool(name="work", bufs=2) as work,
            tc.tile_pool(name="outp", bufs=1) as outp,
        ):
            # mrg is tiny (32KB), first on the scalar ring; NOT via
            # gpsimd SWDGE - any SWDGE DMA degrades SDMA engine 15 for
            # the whole run (descriptor-ring AXI contention).
            m_sb = consts.tile([P, E], F32)
            nc.scalar.dma_start(m_sb[:], m_ext[:])

            # Primers: walrus allows only ONE sync-wait per compute
            # instruction. Give every engine a first op with no other
            # dependency, and absorb the mrg-DMA sem into a throwaway
            # PE op.
            prim = consts.tile([P, 4], F32)
            nc.scalar.copy(prim[:, 1:2], nc.const_aps.tensor(1.0, (P, 1)))
            nc.gpsimd.memset(prim[:, 2:3], 0.0)
            # PE warm-up burst on a memset scratch so the HAM clock-gate
            # reaches K=8/8 before the real GEMM starts; alternate two
            # PSUM banks (same-bank WAW serializes the PE).
            wsc = consts.tile([P, 4 * P], F16)
            nc.vector.memset(wsc[:], 0.0)
            scr = ps_misc.tile([P, BW], F32)
            scr2 = ps_misc.tile([P, BW], F32)
            for k in range(8):
                nc.tensor.matmul([scr, scr2][k % 2][:], wsc[:, 0:P],
                                 wsc[:], start=True, stop=True)

            # all 16 input DMAs up front, alternating the two HWDGE
            # rings; both rings open with x (a const DMA ahead of the
            # first x delays it ~10us: data + ring completion gap).
            # mrg (32KB) slots in after sync's first x: lands ~14us,
            # needed at ~21us, and its ring gap hides behind the other
            # ring's data mid-stream.
            xts = {}
            for s in range(NB):
                for g in range(NG):
                    j = NG * s + g
                    if s == 0:
                        xt = xpool0.tile([P, GC, BW + P], F16, name="x0t",
                                         tag="x0t")
                        src = x0_ext[g]
                    else:
                        xt = xpool.tile([P, GC, BW], F16, name="xt",
                                        tag="xt")
                        src = x_ext[j - NG]
                    dma = nc.sync.dma_start if j % 2 == 0 \
                        else nc.scalar.dma_start
                    dma(xt[:], src)
                    xts[(s, g)] = xt

            def wsl(c):
                # (hi|lo) stationary for chunk c, folded in block-0 tiles
                return xts[(0, c // GC)][:, c % GC, BW:BW + P]

            mx_all = outp.tile([P, NT, 8], F32)
            ix_all = outp.tile([P, NT, 8], U32)
            gap = outp.tile([P, NT, 1], F32)
            ow_all = outp.tile([P, NT, 2], F32)
            oi_all = outp.tile([P, NT, 2], U32)

            lt_sbs = {}

            def backend_copy(s, lt_ps):
                # two half copies: the first merge pair only needs the
                # first 256 columns
                lt_sb = work.tile([P, BW], F32, name="lt_sb", tag="lt_sb")
                hw = BW // 2
                nc.scalar.copy(lt_sb[:, 0:hw], lt_ps[:, 0:hw])
                nc.scalar.copy(lt_sb[:, hw:BW], lt_ps[:, hw:BW])
                lt_sbs[s] = lt_sb

            def backend_rest(s):
                # merge+transpose: lg[tok, e] = ltT_blk.T @ [I64; I64]
                lt_sb = lt_sbs.pop(s)
                # alternate merge target banks: independent groups in
                # the SAME bank serialize (WAW on the bank clear); two
                # banks pipeline and each max8 starts after its own pair
                lgs = [ps_lg.tile([P, TPB // 2, E], F32, name="lg",
                                  tag="lg") for _ in range(2)]
                for b in range(TPB):
                    nc.tensor.matmul(
                        lgs[b % 2][:, b // 2, :],
                        lt_sb[:, b * P:(b + 1) * P],
                        m_sb[:],
                        start=(b < 2), stop=(b >= TPB - 2),
                    )
                for b in range(TPB):
                    t = s * TPB + b
                    nc.vector.max(mx_all[:, t, :], lgs[b % 2][:, b // 2, :])
                sl = slice(s * TPB, (s + 1) * TPB)
                # weights path (ACT) only needs the max VALUES; emit it
                # before the index chain so the two run in parallel
                nc.vector.scalar_tensor_tensor(
                    gap[:, sl, :], mx_all[:, sl, 0:1], 1.0,
                    mx_all[:, sl, 1:2],
                    op0=mybir.AluOpType.mult, op1=mybir.AluOpType.subtract,
                )
                nc.scalar.activation(
                    ow_all[:, sl, 0:1], gap[:, sl, :],
                    mybir.ActivationFunctionType.Sigmoid,
                )
                nc.scalar.activation(
                    ow_all[:, sl, 1:2], gap[:, sl, :],
                    mybir.ActivationFunctionType.Sigmoid, scale=-1.0,
                )
                for b in range(TPB):
                    t = s * TPB + b
                    nc.vector.max_index(ix_all[:, t, :], mx_all[:, t, :],
                                        lgs[b % 2][:, b // 2, :])
                nc.gpsimd.tensor_copy(oi_all[:, sl, :], ix_all[:, sl, 0:2])

            # per-block GEMM chase: block s's h-accumulation completes at
            # (s+1)/4 of the stream; its backend overlaps block s+1's
            # DMAs/GEMMs. Merges deferred one block so the PSUM->SBUF
            # copy hides behind the next block's first GEMM group.
            for s in range(NB):
                lt_ps = ps_lt.tile([P, BW], F32, name="lt", tag="lt")
                # group order [0, 2, 1, 3]: the scalar ring (odd DMAs,
                # behind mrg) runs ~1.5 slots behind the sync ring, so
                # consume in arrival order - three short PE waits keep
                # the HAM window busy instead of one ~6us cold-trigger.
                for gi, g in enumerate((0, 2, 1, 3)):
                    for k in range(GC):
                        nc.tensor.matmul(
                            lt_ps[:],
                            wsl(GC * g + k),
                            xts[(s, g)][:, k, 0:BW],
                            start=(gi == 0 and k == 0),
                            stop=(gi == NG - 1 and k == GC - 1),
                        )
                    if gi == 0 and s > 0:
                        backend_rest(s - 1)
                backend_copy(s, lt_ps)
            backend_rest(NB - 1)

            nc.sync.dma_start(ow_ext[:], ow_all[:])
            nc.scalar.dma_start(oi_ext[:], oi_all[:])

    return nc


_NC_CACHE = {}


def _get_nc():
    if "nc" not in _NC_CACHE:
        _NC_CACHE["nc"] = build_nc()
    return _NC_CACHE["nc"]


def make_in_maps(x: np.ndarray, gate_w: np.ndarray):
    """Shard full inputs into per-core input maps (host-side layout +
    fp16 cast; not on the device critical path)."""
    xf = x.reshape(TOK_TOTAL, H)
    # [core, blk, tok, grp, chk, p] -> [core, blk, grp, p, chk, tok]
    # h = g*(GC*P) + k*P + p
    xq = xf.reshape(N_CORES, NB, BW, NG, GC, P).astype(np.float16)
    xq = xq.transpose(0, 1, 3, 5, 4, 2)
    # gate weight hi/lo fp16 split: whl[p, c, 0:64]=hi, [p, c, 64:128]=lo
    w_hi = gate_w.astype(np.float16)
    w_lo = (gate_w - w_hi.astype(np.float32)).astype(np.float16)
    wh = w_hi.T.reshape(NCH, P, E).transpose(1, 0, 2)
    wl = w_lo.T.reshape(NCH, P, E).transpose(1, 0, 2)
    whl = np.concatenate([wh, wl], axis=2)          # [P, NCH, P]
    wg = whl.reshape(P, NG, GC, P).transpose(1, 0, 2, 3)  # [NG, P, GC, P]
    mrg = np.ascontiguousarray(
        np.vstack([np.eye(E), np.eye(E)]).astype(np.float32))
    return [
        {"x0": np.ascontiguousarray(
            np.concatenate([xq[i, 0], wg], axis=3)),
         "x": np.ascontiguousarray(
             xq[i, 1:].reshape((NB - 1) * NG, P, GC, BW)),
         "mrg": mrg}
        for i in range(N_CORES)
    ]


def kernel(x, gate_w, _trace: bool = False):
    x = np.asarray(x, dtype=np.float32)
    gate_w = np.asarray(gate_w, dtype=np.float32)
    nc = _get_nc()
    in_maps = make_in_maps(x, gate_w)
    res = run_bass_kernel_spmd(
        nc, in_maps, core_ids=list(range(N_CORES)), trace=_trace
    )
    out_w = np.concatenate(
        [res.results[i]["out_w"].transpose(1, 0, 2).reshape(TOK, 2)
         for i in range(N_CORES)])
    out_i = np.concatenate(
        [res.results[i]["out_i"].transpose(1, 0, 2).reshape(TOK, 2)
         for i in range(N_CORES)])
    topk_weights = out_w.reshape(B, S, 2)
    topk_indices = out_i.astype(np.int32).reshape(B, S, 2)
    if _trace:
        kernel._last_result = res
    return topk_weights, topk_indices
